# revision 35
# baseline (speedup 1.0000x reference)
"""2-layer GAT encoder on 8 Trainium2 NeuronCores.

Sharding: destination nodes (and their incoming edges) are partitioned across
the 8 cores (6250 dst nodes each).  Edges are sorted by dst on the host,
grouped into windows of 128 dst nodes, split into a low-src / high-src run
(int16 gather indices), and padded to a fixed tile count so every core runs
the identical SPMD program.

Four device launches; between launches the host does only index-space work
(shard/concat/transpose, and per-edge fancy-indexing of the tiny per-node
attention-coefficient tables into streamed per-edge arrays):

  A: xp1 = x @ W1ext        (per-core node shard)  -> feature rows + a_src/a_dst
  B: layer-1 edge phase     (gather + segment softmax reduction) -> h shard
  C: xp2 = h @ W2ext        (per-core node shard)
  D: layer-2 edge phase     -> out shard

The edge phase gathers 512B source-feature rows per edge with dma_gather,
builds a weighted one-hot per 128-edge tile (one tensor_scalar is_equal*mult
per head) and segment-reduces via PSUM-accumulating matmuls; denominators come
from ones-column matmuls; the epilogue divides, adds bias and applies ELU.
"""

import numpy as np
from dataclasses import dataclass

import concourse.bass as bass
import concourse.bacc as bacc
import concourse.tile as tile
import concourse.mybir as mybir
from concourse.bass_utils import run_bass_kernel_spmd

AF = mybir.ActivationFunctionType
ALU = mybir.AluOpType
F32 = mybir.dt.float32
I16 = mybir.dt.int16

SPLIT = 32768  # int16 gather-index limit -> low/high table split
ABLATE_MASK = False     # cost-model ablation: skip mask builds
ABLATE_GATHER = False   # cost-model ablation: skip gathers
ABLATE_MM = False       # cost-model ablation: skip matmuls


@dataclass
class GatCfg:
    n: int = 50000
    d_in: int = 128
    c1: int = 64
    c2: int = 32
    n_cores: int = 8
    wwin: int = 128
    gchunk: int = 8   # tiles per dma_gather (HW SWDGE ring holds ~1024 descs)

    @property
    def ns(self):
        return self.n // self.n_cores

    @property
    def nwin(self):
        return (self.ns + self.wwin - 1) // self.wwin


def _wrap_idx(idx):
    """[num] int32 -> [128, num/16] int16 dma_gather layout (16-wrap, x8)."""
    num = idx.shape[0]
    assert num % 16 == 0
    w = idx.reshape(num // 16, 16).T.astype(np.int16)   # [16, num/16]
    return np.ascontiguousarray(np.tile(w, (8, 1)))      # [128, num/16]


def _host_prep(cfg: GatCfg, edge_index):
    """Shard + degree-balanced windowing + low/high-src split + pad.

    Destinations are bin-packed into windows (least-loaded-first on total
    degree) so the fixed per-window tile counts TwA/TwB carry minimal padding.
    Returns (TwA, TwB, dloc, esrc, edst, gixA, gixB, dstmap); dstmap[c, i] is
    the in-shard dst id whose result lands in output row i of core c.
    """
    import heapq
    n, ns, wwin, nwin = cfg.n, cfg.ns, cfg.wwin, cfg.nwin
    loops = np.arange(n, dtype=np.int64)
    src = np.concatenate([edge_index[0].astype(np.int64), loops])
    dst = np.concatenate([edge_index[1].astype(np.int64), loops])

    core_of = dst // ns
    din = dst % ns
    # per-core degree-balanced window assignment
    win_of = np.zeros((cfg.n_cores, ns), dtype=np.int64)   # dst -> window
    slot_of = np.zeros((cfg.n_cores, ns), dtype=np.int64)  # dst -> slot in win
    dstmap = np.zeros((cfg.n_cores, ns), dtype=np.int64)
    for c in range(cfg.n_cores):
        deg = np.bincount(din[core_of == c], minlength=ns)
        order_d = np.argsort(-deg, kind="stable")
        heap = [(0, w, 0) for w in range(nwin)]  # (load, win, count)
        cap = [wwin] * nwin
        # last window may be short (ns % wwin)
        cap[nwin - 1] = ns - (nwin - 1) * wwin
        counts = [0] * nwin
        heapq.heapify(heap)
        stash = []
        for d in order_d:
            while True:
                load, w, cnt = heapq.heappop(heap)
                if counts[w] < cap[w]:
                    break
            sl = counts[w]
            counts[w] += 1
            win_of[c, d] = w
            slot_of[c, d] = sl
            dstmap[c, w * wwin + sl] = d
            heapq.heappush(heap, (load + deg[d], w, counts[w]))

    gwin_raw = core_of * nwin + win_of[core_of, din]
    order = np.lexsort((src >= SPLIT, gwin_raw))
    src_s, dst_s = src[order], dst[order]
    gwin = gwin_raw[order]
    low = src_s < SPLIT
    ngrp = cfg.n_cores * nwin
    cnt_lo = np.bincount(gwin[low], minlength=ngrp)
    cnt_hi = np.bincount(gwin[~low], minlength=ngrp)
    TwA = int(np.ceil(cnt_lo.max() / 128))
    TwB = int(np.ceil(cnt_hi.max() / 128))
    Tw = TwA + TwB

    gidx = np.zeros((cfg.n_cores, nwin, Tw * 128), dtype=np.int32)
    dloc = np.full((cfg.n_cores, nwin, Tw * 128), -1.0, dtype=np.float32)
    esrc = np.zeros((cfg.n_cores, nwin, Tw * 128), dtype=np.int32)
    edst = np.zeros((cfg.n_cores, nwin, Tw * 128), dtype=np.int32)

    starts_all = np.concatenate([[0], np.cumsum(np.bincount(gwin, minlength=ngrp))])
    for c in range(cfg.n_cores):
        for w in range(nwin):
            g = c * nwin + w
            s0, s1 = starts_all[g], starts_all[g + 1]
            nlo = cnt_lo[g]
            for base, a, b, off in ((0, s0, s0 + nlo, 0),
                                    (TwA * 128, s0 + nlo, s1, SPLIT)):
                m = b - a
                if m == 0:
                    continue
                sl = slice(base, base + m)
                gidx[c, w, sl] = src_s[a:b] - off
                dloc[c, w, sl] = slot_of[c, dst_s[a:b] % ns].astype(np.float32)
                esrc[c, w, sl] = src_s[a:b]
                edst[c, w, sl] = dst_s[a:b]

    gixA = np.zeros((cfg.n_cores, nwin, 128, TwA * 8), dtype=np.int16)
    gixB = np.zeros((cfg.n_cores, nwin, 128, TwB * 8), dtype=np.int16)
    for c in range(cfg.n_cores):
        for w in range(nwin):
            gixA[c, w] = _wrap_idx(gidx[c, w, :TwA * 128])
            gixB[c, w] = _wrap_idx(gidx[c, w, TwA * 128:])

    def slots(arr):
        return np.ascontiguousarray(
            arr.reshape(cfg.n_cores, nwin, Tw, 128).transpose(0, 1, 3, 2))

    return TwA, TwB, slots(dloc), slots(esrc), slots(edst), gixA, gixB, dstmap


# --------------------------------------------------------------------------
# launch builders
# --------------------------------------------------------------------------

def build_table_kernel(nc, cfg: GatCfg, feat2, name):
    """xp = xT_shard.T @ Wext; emits feature rows + al_src + al_dst tables.

    feat2: total feature columns (2*c). Wext has feat2+4 columns.
    Outputs are staged GRP tiles at a time to amortize DMA fixed costs.
    """
    ns = cfg.ns
    wcols = feat2 + 4
    GRP = 8
    xT = nc.dram_tensor("xT", [cfg.d_in, ns], F32, kind="ExternalInput")
    we = nc.dram_tensor("we", [cfg.d_in, wcols], F32, kind="ExternalInput")
    xp = nc.dram_tensor("xp", [ns, feat2], F32, kind="ExternalOutput")
    als = nc.dram_tensor("als", [ns, 2], F32, kind="ExternalOutput")
    ald = nc.dram_tensor("ald", [ns, 2], F32, kind="ExternalOutput")

    with tile.TileContext(nc) as tc:
        with (
            tc.tile_pool(name="c", bufs=1) as cpool,
            tc.tile_pool(name="x", bufs=3) as xpool,
            tc.tile_pool(name="ps", bufs=8, space="PSUM") as pspool,
            tc.tile_pool(name="o", bufs=2) as opool,
        ):
            ws = cpool.tile([128, wcols], F32)
            nc.sync.dma_start(ws[:], we.ap()[:, :])
            nfull = ns // 128            # full 128-row tiles
            rem = ns - nfull * 128
            BLK = GRP * 128
            k = 0
            for g0 in range(0, nfull * 128, BLK):
                gn = min(GRP, nfull - g0 // 128)
                bsz = gn * 128
                xt = xpool.tile([128, BLK], F32, tag="xt")
                nc.sync.dma_start(xt[:, :bsz], xT.ap()[:, g0:g0 + bsz])
                st = opool.tile([128, GRP, wcols], F32, tag="st")
                for g in range(gn):
                    ps = pspool.tile([128, wcols], F32, tag="ps")
                    nc.tensor.matmul(ps[:, :], xt[:, g * 128:(g + 1) * 128],
                                     ws[:], start=True, stop=True)
                    if k % 2 == 0:
                        nc.vector.tensor_copy(st[:, g, :], ps[:, :])
                    else:
                        nc.scalar.copy(st[:, g, :], ps[:, :])
                    k += 1
                nc.sync.dma_start(
                    xp.ap()[g0:g0 + bsz, :].rearrange("(g p) c -> p g c", p=128),
                    st[:, :gn, 0:feat2])
                nc.sync.dma_start(
                    als.ap()[g0:g0 + bsz, :].rearrange("(g p) c -> p g c", p=128),
                    st[:, :gn, feat2:feat2 + 2])
                nc.sync.dma_start(
                    ald.ap()[g0:g0 + bsz, :].rearrange("(g p) c -> p g c", p=128),
                    st[:, :gn, feat2 + 2:feat2 + 4])
            if rem:
                r0 = nfull * 128
                xt = xpool.tile([128, BLK], F32, tag="xt")
                nc.sync.dma_start(xt[:, :rem], xT.ap()[:, r0:r0 + rem])
                ps = pspool.tile([128, wcols], F32, tag="ps")
                nc.tensor.matmul(ps[:rem, :], xt[:, :rem], ws[:],
                                 start=True, stop=True)
                ob = opool.tile([128, GRP, wcols], F32, tag="st")
                nc.vector.tensor_copy(ob[:rem, 0, :], ps[:rem, :])
                nc.sync.dma_start(xp.ap()[r0:r0 + rem, :], ob[:rem, 0, 0:feat2])
                nc.sync.dma_start(als.ap()[r0:r0 + rem, :],
                                  ob[:rem, 0, feat2:feat2 + 2])
                nc.sync.dma_start(ald.ap()[r0:r0 + rem, :],
                                  ob[:rem, 0, feat2 + 2:feat2 + 4])
    return nc


def build_edge_kernel(nc, cfg: GatCfg, TwA, TwB, cdim, out_cols, name):
    """Edge phase for one layer.  cdim = per-head dim (64 / 32)."""
    ns, nwin, ww = cfg.ns, cfg.nwin, cfg.wwin
    Tw = TwA + TwB
    feat2 = 2 * cdim

    xp = nc.dram_tensor("xp", [cfg.n, feat2], F32, kind="ExternalInput")
    gix = nc.dram_tensor("gix", [nwin, 128, Tw * 8], I16, kind="ExternalInput")
    mrg = nc.dram_tensor("mrg", [nwin, 128, 5 * Tw], F32, kind="ExternalInput")
    bb = nc.dram_tensor("bb", [128, out_cols], F32, kind="ExternalInput")
    iot = nc.dram_tensor("iot", [128, ww], F32, kind="ExternalInput")
    out = nc.dram_tensor("out", [ns, out_cols], F32, kind="ExternalOutput")

    with tile.TileContext(nc) as tc:
        with (
            tc.tile_pool(name="c", bufs=1) as cpool,
            tc.tile_pool(name="i", bufs=4) as ipool,
            tc.tile_pool(name="x", bufs=3) as xpool,
            tc.tile_pool(name="w", bufs=4) as wpool,
            tc.tile_pool(name="m", bufs=8) as mpool,
            tc.tile_pool(name="ps", bufs=2, space="PSUM") as pspool,
            tc.tile_pool(name="e", bufs=2) as epool,
        ):
            bs = cpool.tile([128, out_cols], F32)
            nc.sync.dma_start(bs[:], bb.ap()[:, :])
            ios = cpool.tile([128, ww], F32)
            nc.sync.dma_start(ios[:], iot.ap()[:, :])
            ones2 = cpool.tile([128, 2], F32)
            nc.vector.memset(ones2[:], 1.0)

            for w in range(nwin):
                gx = ipool.tile([128, Tw * 8], I16, tag="gx")
                nc.sync.dma_start(gx[:], gix.ap()[w])
                mg = wpool.tile([128, 5 * Tw], F32, tag="mg")
                nc.sync.dma_start(mg[:], mrg.ap()[w])
                dl = mg[:, 0:Tw]
                sv = mg[:, Tw:3 * Tw].rearrange("p (t h) -> p t h", h=2)
                ad = mg[:, 3 * Tw:5 * Tw].rearrange("p (t h) -> p t h", h=2)

                X = xpool.tile([128, Tw, feat2], F32, tag="X")
                for t0_, nt_, c0_, tab in (
                    (0, TwA, 0, xp.ap()[0:SPLIT, :]),
                    (TwA, TwB, TwA * 8, xp.ap()[SPLIT:cfg.n, :]),
                ):
                    for cb in range(0, nt_, cfg.gchunk):
                        ct = min(cfg.gchunk, nt_ - cb)
                        if ABLATE_GATHER:
                            continue
                        nc.gpsimd.dma_gather(
                            X[:, t0_ + cb:t0_ + cb + ct, :], tab,
                            gx[:, c0_ + cb * 8:c0_ + (cb + ct) * 8],
                            num_idxs=ct * 128, num_idxs_reg=ct * 128,
                            elem_size=feat2)

                # w = exp(leakyrelu(al_src + al_dst, 0.2))
                nc.vector.tensor_tensor(sv[:], sv[:], ad[:], ALU.add)
                s2 = wpool.tile([128, Tw, 2], F32, tag="s2")
                nc.vector.tensor_scalar(s2[:], sv[:], 0.2, None, ALU.mult)
                nc.vector.tensor_tensor(sv[:], sv[:], s2[:], ALU.max)
                wt = wpool.tile([128, Tw, 2], F32, tag="wt")
                nc.scalar.activation(wt[:], sv[:], AF.Exp)
                # head-1/head-0 weight ratio: m1 = m0 * rr folds into the rhs
                rr = wpool.tile([128, Tw], F32, tag="rr")
                nc.vector.tensor_tensor(rr[:], sv[:, :, 1], sv[:, :, 0],
                                        ALU.subtract)
                nc.scalar.activation(rr[:], rr[:], AF.Exp)

                ps0 = pspool.tile([128, cdim], F32, tag="ps0")
                ps1 = pspool.tile([128, cdim], F32, tag="ps1")
                pd0 = pspool.tile([128, 2], F32, tag="pd0")
                pd1 = pspool.tile([128, 1], F32, tag="pd1")
                for t in range(Tw):
                    st, sp = (t == 0), (t == Tw - 1)
                    m0 = mpool.tile([128, ww], F32, tag="m0")
                    if not ABLATE_MASK:
                        nc.vector.tensor_scalar(
                            m0[:], ios[:, :], dl[:, t:t + 1], wt[:, t, 0:1],
                            ALU.is_equal, ALU.mult)
                    if not ABLATE_MASK:
                        nc.vector.tensor_scalar(
                            X[:, t, cdim:feat2], X[:, t, cdim:feat2],
                            rr[:, t:t + 1], None, ALU.mult)
                    if not ABLATE_MM:
                        nc.tensor.matmul(ps0[:ww, :], m0[:], X[:, t, 0:cdim],
                                         start=st, stop=sp)
                        nc.tensor.matmul(pd0[:ww, :], m0[:], ones2[:],
                                         start=st, stop=sp)
                        nc.tensor.matmul(ps1[:ww, :], m0[:], X[:, t, cdim:feat2],
                                         start=st, stop=sp)
                        nc.tensor.matmul(pd1[:ww, :], m0[:], rr[:, t:t + 1],
                                         start=st, stop=sp)

                # epilogue: divide, +bias, ELU, store rows
                wd = min(ww, ns - w * ww)
                den = epool.tile([128, 2], F32, tag="den")
                nc.vector.tensor_scalar(den[:ww, 0:1], pd0[:ww, 0:1],
                                        1e-30, None, ALU.max)
                nc.vector.tensor_scalar(den[:ww, 1:2], pd1[:ww, 0:1],
                                        1e-30, None, ALU.max)
                rcp = epool.tile([128, 2], F32, tag="rcp")
                nc.vector.reciprocal(rcp[:ww, :], den[:ww, :])
                V = epool.tile([128, out_cols], F32, tag="V")
                nc.vector.tensor_scalar(V[:ww, 0:cdim], ps0[:ww, :],
                                        rcp[:ww, 0:1], None, ALU.mult)
                nc.vector.tensor_scalar(V[:ww, cdim:feat2], ps1[:ww, :],
                                        rcp[:ww, 1:2], None, ALU.mult)
                nc.vector.tensor_tensor(V[:ww, :], V[:ww, :], bs[:ww, :], ALU.add)
                E = epool.tile([128, out_cols], F32, tag="E")
                nc.vector.tensor_scalar(E[:ww, :], V[:ww, :], 0.0, None, ALU.min)
                nc.scalar.activation(E[:ww, :], E[:ww, :], AF.Exp)
                nc.vector.tensor_scalar(E[:ww, :], E[:ww, :], -1.0, None, ALU.add)
                H = epool.tile([128, out_cols], F32, tag="H")
                nc.vector.tensor_tensor(H[:ww, :], V[:ww, :], E[:ww, :], ALU.max)
                nc.sync.dma_start(out.ap()[w * ww:w * ww + wd, :], H[:wd, :])
    return nc


# --------------------------------------------------------------------------
# host orchestration
# --------------------------------------------------------------------------

def _ext_w(W, a_s, a_d, c):
    """[d, 2c+4] = [W | W_lo@a_s0 | W_hi@a_s1 | W_lo@a_d0 | W_hi@a_d1]."""
    return np.ascontiguousarray(np.concatenate([
        W,
        (W[:, :c] @ a_s[0])[:, None], (W[:, c:] @ a_s[1])[:, None],
        (W[:, :c] @ a_d[0])[:, None], (W[:, c:] @ a_d[1])[:, None],
    ], axis=1), dtype=np.float32)


SIM_MODE = False  # set True to run launches in CoreSim instead of hardware


class _SimRes:
    def __init__(self, results):
        self.results = results
        self.exec_time_ns = None


class _Launch:
    def __init__(self, nc, cfg):
        self.nc = nc
        self.cfg = cfg

    def run(self, in_maps, trace=False):
        if SIM_MODE:
            from concourse.bass_interp import MultiCoreSim
            sim = MultiCoreSim(self.nc, num_cores=self.cfg.n_cores, trace=False,
                               require_finite=False, require_nnan=False)
            cores = list(sim.cores.values())
            for c, core in enumerate(cores):
                for k, v in in_maps[c].items():
                    core.tensor(k)[:] = v
            sim.simulate(check_with_hw=False)
            outs = []
            for core in cores:
                d = {}
                for alloc in self.nc.m.functions[0].allocations:
                    if (isinstance(alloc, mybir.MemoryLocationSet)
                            and alloc.kind == "ExternalOutput"):
                        nm = alloc.memorylocations[0].name
                        d[nm] = np.array(core.tensor(nm))
                outs.append(d)
            return _SimRes(outs)
        res = run_bass_kernel_spmd(self.nc, in_maps,
                                   core_ids=list(range(self.cfg.n_cores)),
                                   trace=trace)
        return res


def prepare(x, edge_index, W1, a_src1, a_dst1, b1, W2, a_src2, a_dst2, b2,
            cfg=None):
    x = np.asarray(x, dtype=np.float32)
    cfg = cfg or GatCfg()
    TwA, TwB, dloc, esrc, edst, gixA, gixB, dstmap = _host_prep(
        cfg, np.asarray(edge_index))
    inv = np.zeros_like(dstmap)
    for c in range(cfg.n_cores):
        inv[c, dstmap[c]] = np.arange(cfg.ns)
    Tw = TwA + TwB
    ncore, ns, nwin = cfg.n_cores, cfg.ns, cfg.nwin

    w1e = _ext_w(np.asarray(W1, np.float32), np.asarray(a_src1, np.float32),
                 np.asarray(a_dst1, np.float32), cfg.c1)
    w2e = _ext_w(np.asarray(W2, np.float32), np.asarray(a_src2, np.float32),
                 np.asarray(a_dst2, np.float32), cfg.c2)
    b1b = np.ascontiguousarray(
        np.broadcast_to(np.asarray(b1, np.float32)[None, :], (128, 2 * cfg.c1)))
    b2b = np.ascontiguousarray(
        np.broadcast_to(np.asarray(b2, np.float32)[None, :], (128, 2 * cfg.c2)))
    iot = np.ascontiguousarray(
        np.broadcast_to(np.arange(cfg.wwin, dtype=np.float32)[None, :],
                        (128, cfg.wwin)))

    # ---- build + compile the four programs ----
    def mk(builder, *args):
        nc = bacc.Bacc("TRN2", num_devices=ncore, debug=False)
        builder(nc, *args)
        nc.compile()
        return _Launch(nc, cfg)

    LA = mk(build_table_kernel, cfg, 2 * cfg.c1, "t1")
    LB = mk(build_edge_kernel, cfg, TwA, TwB, cfg.c1, 2 * cfg.c1, "e1")
    LC = mk(build_table_kernel, cfg, 2 * cfg.c2, "t2")
    LD = mk(build_edge_kernel, cfg, TwA, TwB, cfg.c2, 2 * cfg.c2, "e2")
    Tw = TwA + TwB
    ncore_r = range(ncore)
    gix_m = [np.ascontiguousarray(np.concatenate([gixA[c], gixB[c]], axis=2))
             for c in ncore_r]

    def mk_mrg(c, als, ald):
        nwin_ = cfg.nwin
        return np.ascontiguousarray(np.concatenate([
            dloc[c],
            als[esrc[c]].reshape(nwin_, 128, 2 * Tw),
            ald[edst[c]].reshape(nwin_, 128, 2 * Tw),
        ], axis=2))

    def run_all(trace=False):
        exec_ns = []

        def _t(res):
            if res.exec_time_ns is not None:
                exec_ns.append(res.exec_time_ns)

        # A: layer-1 tables (node-sharded)
        inA = [{"xT": np.ascontiguousarray(x[c * ns:(c + 1) * ns].T),
                "we": w1e} for c in range(ncore)]
        rA = LA.run(inA, trace)
        _t(rA)
        xp1 = np.concatenate([rA.results[c]["xp"] for c in range(ncore)])
        als1 = np.concatenate([rA.results[c]["als"] for c in range(ncore)])
        ald1 = np.concatenate([rA.results[c]["ald"] for c in range(ncore)])

        # B: layer-1 edge phase
        inB = [{"xp": xp1, "gix": gix_m[c], "mrg": mk_mrg(c, als1, ald1),
                "bb": b1b, "iot": iot} for c in range(ncore)]
        rB = LB.run(inB, trace)
        _t(rB)
        h = np.concatenate([rB.results[c]["out"][inv[c]] for c in range(ncore)])

        # C: layer-2 tables
        inC = [{"xT": np.ascontiguousarray(h[c * ns:(c + 1) * ns].T),
                "we": w2e} for c in range(ncore)]
        rC = LC.run(inC, trace)
        _t(rC)
        xp2 = np.concatenate([rC.results[c]["xp"] for c in range(ncore)])
        als2 = np.concatenate([rC.results[c]["als"] for c in range(ncore)])
        ald2 = np.concatenate([rC.results[c]["ald"] for c in range(ncore)])

        # D: layer-2 edge phase
        inD = [{"xp": xp2, "gix": gix_m[c], "mrg": mk_mrg(c, als2, ald2),
                "bb": b2b, "iot": iot} for c in range(ncore)]
        rD = LD.run(inD, trace)
        _t(rD)
        out = np.concatenate([rD.results[c]["out"][inv[c]]
                              for c in range(ncore)])
        total_ns = sum(exec_ns) if len(exec_ns) == 4 else None
        return out, total_ns

    return cfg, (TwA, TwB), run_all


def kernel(x, edge_index, W1, a_src1, a_dst1, b1, W2, a_src2, a_dst2, b2):
    x = np.asarray(x, dtype=np.float32)
    _, _, run_all = prepare(x, edge_index, W1, a_src1, a_dst1, b1,
                            W2, a_src2, a_dst2, b2)
    out, _ = run_all()
    return out, x
